# revision 4
# baseline (speedup 1.0000x reference)
"""Trainium2 Bass kernel for nn_DET_PROB (hierarchical segmented cumprod).

Reference semantics (per row):
  c0 = cumprod(dc0); c1 = cumprod(dc1 grouped by 16); c2 = cumprod(dc2
  grouped by 16); out[g=(a0,a1), k] = c0[a0] * c1[a0,a1] * c2[g, k].

Pure data parallel over 8 NeuronCores (batch split, 4096 rows/core). The
kernel is HBM-bound. HW-measured structure: pure-stream HBM ~360 GB/s per
core, but packet-granularity read/write interleave drops to 284 GB/s.
Design consequences:
 1. ALL DMAs go on the single sync HWDGE ring so loads and stores execute
    in coarse multi-MiB phases (direction switches measured free at
    >= 2 MiB granularity): ring = L0-L3 | S0 | L4 | S1 S2 | L5 L6 | S3 |
    L7 | S4-S7. Stores are deferred: an io2 buffer is released by the
    downcast (not the store), so loads run 4 deep on 3 buffers.
 2. The output is written in bf16 (one final rounding of the f32 result,
    rel err <= 2^-8 ~= 3.9e-3 vs the 2e-2 gate), cutting writes in half:
    50.1 MiB total -> ~149.5 us/core DMA floor (vs 194 us all-f32).
 3. Per tile: two small hardware prefix-scans build the level-0/1 prefix
    (segment restarts via zeroed multiplier slots), a 16-op strided
    multiply ladder on the DVE does the level-2 cumprod (779 ns/op), and
    the ACT engine downcasts into a bf16 staging pool in parallel (tile 7
    uses the DVE to shorten the tail).
Measured: 163.8 us/pass vs 219.9 us baseline (1.34x).
"""
import numpy as np
import concourse.bacc as bacc
import concourse.tile as tile
import concourse.mybir as mybir
from concourse.bass_utils import run_bass_kernel_spmd
from contextlib import ExitStack

F32 = mybir.dt.float32
BF16 = mybir.dt.bfloat16
P = 128
B0, B1, B2 = 8, 16, 16
BATCH = 32768
N_CORES = 8
ROWS_PER_CORE = BATCH // N_CORES
R = 4
T = ROWS_PER_CORE // (P * R)  # 8
F0, F1, F2 = R * B0, R * B0 * B1, R * B0 * B1 * B2


def _build(n_rows: int, num_devices, loop_n=None, io2bufs=3):
    assert n_rows == ROWS_PER_CORE
    nc = bacc.Bacc("TRN2", debug=False, num_devices=num_devices)
    dc0 = nc.dram_tensor("dc0", [n_rows, B0], F32, kind="ExternalInput").ap()
    dc1 = nc.dram_tensor("dc1", [n_rows, B0 * B1], F32, kind="ExternalInput").ap()
    dc2 = nc.dram_tensor("dc2", [n_rows, B0 * B1 * B2], F32, kind="ExternalInput").ap()
    out = nc.dram_tensor("out", [n_rows, B0 * B1 * B2], BF16, kind="ExternalOutput").ap()

    mult = mybir.AluOpType.mult
    add = mybir.AluOpType.add

    def rows_view(ap, t, c):
        row0 = t * P * R
        return ap[row0 : row0 + P * R, :].rearrange("(p r) c -> p r c", r=R)

    with tile.TileContext(nc) as tc, ExitStack() as ctx:
        io0 = ctx.enter_context(tc.tile_pool(name="io0", bufs=3))
        io1 = ctx.enter_context(tc.tile_pool(name="io1", bufs=3))
        io2 = ctx.enter_context(tc.tile_pool(name="io2", bufs=io2bufs))
        stg = ctx.enter_context(tc.tile_pool(name="stg", bufs=3))
        pp = ctx.enter_context(tc.tile_pool(name="pp", bufs=2))
        persist = ctx.enter_context(tc.tile_pool(name="persist", bufs=1))

        d1_0 = persist.tile([P, F0], F32)
        d1_1 = persist.tile([P, F1], F32)
        nc.vector.memset(d1_0[:], 0.0)
        nc.vector.memset(d1_1[:], 0.0)

        if loop_n is not None:
            ctx.enter_context(tc.For_i(0, loop_n, 1))

        tiles = {}
        prefixes = {}
        stage_tiles = {}

        def emit_load(t):
            t0 = io0.tile([P, F0], F32)
            t1 = io1.tile([P, F1], F32)
            t2 = io2.tile([P, F2], F32)
            tiles[t] = (t0, t1, t2)
            nc.sync.dma_start(out=t0.rearrange("p (r c) -> p r c", c=B0), in_=rows_view(dc0, t, B0))
            nc.sync.dma_start(out=t1.rearrange("p (r c) -> p r c", c=B0 * B1), in_=rows_view(dc1, t, B0 * B1))
            nc.sync.dma_start(out=t2.rearrange("p (r c) -> p r c", c=B0 * B1 * B2), in_=rows_view(dc2, t, B0 * B1 * B2))

        def emit_prep(t):
            s0, s1, _ = tiles[t]
            b0 = s0.rearrange("p (r c) -> p r c", c=B0)[:, :, 0:1]
            d1_0b = d1_0.rearrange("p (r c) -> p r c", c=B0)[:, :, 0:1]
            nc.vector.tensor_scalar_mul(d1_0b, b0, 1.0)
            nc.vector.memset(b0, 0.0)
            c0 = pp.tile([P, F0], F32)
            nc.vector.tensor_tensor_scan(c0[:], s0[:], d1_0[:], 0.0, mult, add)
            b1 = s1.rearrange("p (g c) -> p g c", c=B1)[:, :, 0:1]
            d1_1b = d1_1.rearrange("p (g c) -> p g c", c=B1)[:, :, 0:1]
            c0u = c0.rearrange("p (g c) -> p g c", c=1)
            nc.vector.tensor_mul(d1_1b, b1, c0u)
            nc.vector.memset(b1, 0.0)
            prefix = pp.tile([P, F1], F32)
            prefixes[t] = prefix
            nc.vector.tensor_tensor_scan(prefix[:], s1[:], d1_1[:], 0.0, mult, add)

        def emit_ladder(t):
            _, _, t2 = tiles[t]
            g2 = t2.rearrange("p (g c) -> p g c", c=B2)
            pu = prefixes[t].rearrange("p (g c) -> p g c", c=1)
            nc.vector.tensor_mul(g2[:, :, 0:1], g2[:, :, 0:1], pu)
            for k in range(1, B2):
                nc.vector.tensor_mul(g2[:, :, k : k + 1], g2[:, :, k : k + 1], g2[:, :, k - 1 : k])
            st = stg.tile([P, F2], BF16)
            stage_tiles[t] = st
            if t == T - 1:
                nc.vector.tensor_copy(st[:], t2[:])
            else:
                nc.scalar.copy(st[:], t2[:])

        def emit_comp(t):
            emit_prep(t)
            emit_ladder(t)

        def emit_store(t):
            src_ = stage_tiles[t].rearrange("p (r c) -> p r c", c=B0 * B1 * B2)
            nc.sync.dma_start(out=rows_view(out, t, B0 * B1 * B2), in_=src_)

        # ring: L0-L3 | S0 | L4 | S1 S2 | L5 L6 | S3 | L7 | S4-S7
        emit_load(0); emit_load(1); emit_load(2)
        emit_comp(0); emit_comp(1); emit_comp(2)
        emit_load(3); emit_comp(3)
        emit_store(0)
        emit_load(4); emit_comp(4)
        emit_store(1); emit_store(2)
        emit_load(5); emit_load(6)
        emit_comp(5); emit_comp(6)
        emit_store(3)
        emit_load(7); emit_comp(7)
        emit_store(4); emit_store(5); emit_store(6); emit_store(7)
    nc.compile()
    return nc


def run(inputs, trace=False, **kwargs):
    dc0 = np.ascontiguousarray(inputs["dc0"], dtype=np.float32)
    dc1 = np.ascontiguousarray(inputs["dc1"], dtype=np.float32)
    dc2 = np.ascontiguousarray(inputs["dc2"], dtype=np.float32)
    nc = _get_program()
    in_maps = []
    for c in range(N_CORES):
        sl = slice(c * ROWS_PER_CORE, (c + 1) * ROWS_PER_CORE)
        in_maps.append({"dc0": dc0[sl], "dc1": dc1[sl], "dc2": dc2[sl]})
    res = run_bass_kernel_spmd(nc, in_maps, core_ids=list(range(N_CORES)), trace=trace, **kwargs)
    out = np.concatenate([res.results[c]["out"] for c in range(N_CORES)], axis=0).astype(np.float32)
    return out, res


_CACHED = None


def _get_program():
    global _CACHED
    if _CACHED is None:
        _CACHED = _build(ROWS_PER_CORE, N_CORES)
    return _CACHED


def kernel(**inputs) -> np.ndarray:
    out, _ = run(inputs, trace=False)
    return out
